# revision 5
# baseline (speedup 1.0000x reference)
"""fp8 quantized matmul y = fp8(x) @ fp8(W)^T on 8 Trainium2 NeuronCores.

Reference semantics: x[M,K] f32 and W[N,K] f32 are each cast to
float8_e4m3fn (OCP, round-to-nearest-even) and the matmul accumulates in
fp32.  The cast is a pure element-wise dtype conversion, done here on the
host with ml_dtypes (bit-identical to the reference's jax cast for the
value range involved: |x| < 16, |W| <= 2^-6, both far below 240 where the
OCP and IEEE e4m3 encodings coincide).

Sharding: data-parallel over M — each of the 8 cores computes a
[1024, 4096] slice of y from its x shard; W^T (16 MiB as fp8) is
replicated and lives entirely in SBUF.  The device kernel runs fp8
DoubleRow matmuls (2 contraction rows per PE cell), 16 accumulating
matmuls per [128, 512] PSUM tile, DVE evacuates PSUM -> SBUF, DMA out.

Host-side layouts are pre-transposed so every DMA is a large contiguous
per-partition transfer:
  xt[mt, p, kt, m] = fp8(x_shard)[mt*128 + m, kt*128 + p]   (4 MiB/core)
  wt[nt, p, kt, n] = fp8(W)[nt*512 + n, kt*128 + p]         (16 MiB)
The [p, kt, cols] SBUF tiles feed nc.tensor.matmul sliced
[:, 2t:2t+2, :] — the DoubleRow contraction pair is (kt*128+p) over two
consecutive kt subtiles, identically on both operands.
"""

import numpy as np
import ml_dtypes

P = 128          # partitions
N_CORES = 8
M, K, N = 8192, 4096, 4096
MC = M // N_CORES          # 1024 rows of x per core
MT = MC // P               # 8 m-tiles per core
KT = K // P                # 32 k-subtiles
NB = 512                   # psum bank width (f32)
NT = N // NB               # 8 n-tiles

_NC_CACHE = {}


N_WARMUP = 10  # dummy PE matmuls bridging the startup barrier -> first data


def _emit(nc, tc, mybir, X, W, Y, mt_n, nt_n, kt_n, nb):
    fp8 = mybir.dt.float8e4
    f32 = mybir.dt.float32
    import contextlib

    kq = min(8, kt_n)          # kt-subtiles per W quarter-tile
    wg_n = kt_n // kq          # W groups per nt column
    assert kt_n % kq == 0 and kq % 2 == 0

    with contextlib.ExitStack() as ctx:
        warm = ctx.enter_context(tc.tile_pool(name="warm", bufs=1))
        xpool = ctx.enter_context(tc.tile_pool(name="xpool", bufs=1))
        wpool = ctx.enter_context(tc.tile_pool(name="wpool", bufs=1))
        spool = ctx.enter_context(tc.tile_pool(name="spool", bufs=6))
        wpsum = ctx.enter_context(
            tc.tile_pool(name="wpsum", bufs=1, space="PSUM")
        )
        ppool = ctx.enter_context(
            tc.tile_pool(name="ppool", bufs=4, space="PSUM")
        )

        # PE warmup on memset tiles: occupies the tensor engine from the
        # end of the startup barrier until the first input DMAs land, so
        # the HAM clock gate is released before real matmuls begin.
        wm_x = warm.tile([P, 2, P], fp8, name="wm_x", tag="wm_x")
        wm_w = warm.tile([P, 2, nb], fp8, name="wm_w", tag="wm_w")
        nc.gpsimd.memset(wm_x, 0.0)
        nc.gpsimd.memset(wm_w, 0.0)
        wm_ps = wpsum.tile([P, nb], f32, name="wm_ps", tag="wm_ps")
        for _ in range(N_WARMUP):
            nc.tensor.matmul(
                wm_ps,
                wm_x,
                wm_w,
                start=True,
                stop=True,
                perf_mode=mybir.MatmulPerfMode.DoubleRow,
            )

        # Input loads, all on the SP HWDGE ring, in consumption order:
        # x0, W(nt=0) quarters, x1..x7, then the remaining W quarters.
        xt = [None] * mt_n
        wt = [[None] * wg_n for _ in range(nt_n)]

        def load_x(mt):
            t = xpool.tile([P, kt_n, P], fp8, name=f"xt{mt}", tag=f"xt{mt}")
            nc.sync.dma_start(out=t, in_=X[mt, :, :, :])
            xt[mt] = t

        def load_w(nt, g):
            t = wpool.tile([P, kq, nb], fp8, name=f"wt{nt}_{g}", tag=f"wt{nt}_{g}")
            nc.sync.dma_start(out=t, in_=W[nt, :, g * kq : (g + 1) * kq, :])
            wt[nt][g] = t

        load_x(0)
        for g in range(wg_n):
            load_w(0, g)
        for mt in range(1, mt_n):
            load_x(mt)
        for nt in range(1, nt_n):
            for g in range(wg_n):
                load_w(nt, g)

        n_pairs = kt_n // 2
        pairs_per_g = kq // 2
        for nt in range(nt_n):
            for mt in range(mt_n):
                ps = ppool.tile([P, nb], f32, name="ps", tag="ps")
                for t2 in range(n_pairs):
                    g, lp = divmod(t2, pairs_per_g)
                    nc.tensor.matmul(
                        ps,
                        xt[mt][:, 2 * t2 : 2 * t2 + 2, :],
                        wt[nt][g][:, 2 * lp : 2 * lp + 2, :],
                        start=(t2 == 0),
                        stop=(t2 == n_pairs - 1),
                        perf_mode=mybir.MatmulPerfMode.DoubleRow,
                    )
                st = spool.tile([P, nb], f32, name="st", tag="st")
                nc.vector.tensor_copy(out=st, in_=ps)
                # outputs ride the ACT HWDGE ring so they never queue
                # behind the weight loads on the SP ring
                nc.scalar.dma_start(
                    out=Y[mt * P : (mt + 1) * P, nt * nb : (nt + 1) * nb],
                    in_=st,
                )


def _build(mt_n=MT, nt_n=NT, kt_n=KT, nb=NB, hw=True):
    import concourse.bacc as bacc
    import concourse.mybir as mybir
    import concourse.tile as tile
    from concourse.bass_interp import get_hw_module

    nc = bacc.Bacc("TRN2", target_bir_lowering=False, debug=False)
    X = nc.dram_tensor(
        "xt", [mt_n, P, kt_n, P], mybir.dt.float8e4, kind="ExternalInput"
    ).ap()
    W = nc.dram_tensor(
        "wt", [nt_n, P, kt_n, nb], mybir.dt.float8e4, kind="ExternalInput"
    ).ap()
    Y = nc.dram_tensor(
        "y", [mt_n * P, nt_n * nb], mybir.dt.float32, kind="ExternalOutput"
    ).ap()
    with tile.TileContext(nc) as tc:
        _emit(nc, tc, mybir, X, W, Y, mt_n, nt_n, kt_n, nb)
    nc.compile()
    if hw:
        nc.m = get_hw_module(nc.m)
    return nc


def _get_nc():
    if "nc" not in _NC_CACHE:
        _NC_CACHE["nc"] = _build()
    return _NC_CACHE["nc"]


def _quantize(a):
    # OCP e4m3fn RNE cast (matches jax astype), then reinterpret as the
    # IEEE e4m3 dtype the BIR tensor declares (identical bits below 240).
    return a.astype(ml_dtypes.float8_e4m3fn).view(ml_dtypes.float8_e4m3)


def _in_maps(x, W):
    xq = _quantize(np.ascontiguousarray(x))
    wq = _quantize(np.ascontiguousarray(W))
    # wt[nt, p, kt, n] = wq[nt*NB + n, kt*P + p]
    wt = np.ascontiguousarray(wq.reshape(NT, NB, KT, P).transpose(0, 3, 2, 1))
    maps = []
    for c in range(N_CORES):
        xc = xq[c * MC : (c + 1) * MC]
        # xt[mt, p, kt, m] = xc[mt*P + m, kt*P + p]
        xt = np.ascontiguousarray(xc.reshape(MT, P, KT, P).transpose(0, 3, 2, 1))
        maps.append({"xt": xt, "wt": wt})
    return maps


def _ensure_axon_ntff_hook():
    # Under axon, run_bass_kernel_spmd(trace=True) imports
    # antenv.axon_hooks, which some images lack even though the boot
    # machinery that implements the hook is present.  Register a shim so
    # tracing degrades gracefully instead of raising.
    import sys

    if "antenv.axon_hooks" in sys.modules:
        return
    try:
        from concourse._compat import axon_active

        if not axon_active():
            return
        import importlib.util

        if importlib.util.find_spec("antenv.axon_hooks") is not None:
            return
        import types

        import antenv

        hook = None
        try:
            import trn_agent_boot.trn_boot as _tb

            hook = _tb._ntff_profile_via_ctypes("/opt/axon/libaxon_pjrt.so")
        except Exception:
            hook = None
        mod = types.ModuleType("antenv.axon_hooks")
        mod._hook = hook
        mod.get_axon_ntff_profile_hook = lambda: mod._hook
        def _set(h):
            mod._hook = h
        mod.set_axon_ntff_profile_hook = _set
        antenv.axon_hooks = mod
        sys.modules["antenv.axon_hooks"] = mod
    except Exception:
        pass


def _run(in_maps, trace=False):
    from concourse.bass_utils import run_bass_kernel_spmd

    _ensure_axon_ntff_hook()
    nc = _get_nc()
    return run_bass_kernel_spmd(
        nc, in_maps, core_ids=list(range(len(in_maps))), trace=trace
    )


def kernel(x, W):
    res = _run(_in_maps(x, W))
    return np.concatenate(
        [res.results[c]["y"] for c in range(N_CORES)], axis=0
    ).astype(np.float32, copy=False)
